# revision 6
# baseline (speedup 1.0000x reference)
"""CSI loss kernel v2 for Trainium2 (8 NeuronCores, data parallel).

Math (see reference.py; eps terms dropped where negligible for randn):
  u = |p|, v = |t|.  p2 = a1^2+b1^2 (+1e-30), q2 = a2^2+b2^2 (+1e-30)
  lnp = ln(p2) = 2 ln u;  u = exp(0.5 lnp)   (keeps Act in ONE table)
  cross: cr = a1a2+b1b2, sn = b1a2-a1b2  (p * conj(t) = cr + i sn)
  phase: dth = 2*atan(sn / (uv + cr + eps))   (half-angle, no fixup)
  corr:  cos(dth) = 2/(1+rat^2) - 1  ->  S_IC = sum 1/(1+rat^2)
  js:    R8 = sum u*lnp (=2 sum u ln u), W = sum wt ln wt,
         wt = Sq*u + Sp*v with per-row Sp = sum u, Sq = sum v.

Engine split per chunk [128,2048]:
  Pool: 4 casting DMA loads (f32->bf16, block-batched), phacc/R8/R9 accums
  DVE:  SQSUM/RECIP2/CORR/WT custom ops, TTR accums, bf16 products
  Act:  Ln/Exp (natural_log_exp table) + Arctan (trig table) + Ln(wt)
"""

import numpy as np

import concourse.bass as bass
import concourse.mybir as mybir
from concourse.bass_utils import run_bass_kernel_spmd

AF = mybir.ActivationFunctionType
ALU = mybir.AluOpType
F32 = mybir.dt.float32
BF16 = mybir.dt.bfloat16

B, N = 4096, 4096
NCORES = 8
ROWS_PER_CORE = B // NCORES          # 512
NBLK = ROWS_PER_CORE // 128          # 4 row-blocks of 128
CHUNK = 2048
NCH = N // CHUNK                     # 2 col-chunks per block
NSTAT = 10
S_UU, S_VV, S_U, S_V, S_UV, S_PH, S_IC, S_R8, S_R9, S_W = range(NSTAT)
NCHUNKS = NBLK * NCH                 # 8
ACC_COLS = NCHUNKS * NSTAT           # 80

_ENGINES = ("sync", "vector", "scalar", "gpsimd", "gdma0", "gdma1")
# "gdmaN": DMA ops issued on the gpsimd engine stream but tracked on a
# per-buffer-parity semaphore (DMA completions are async and unordered;
# all waits land on whole-block-set totals).
_STREAM_OF = {"sync": "sync", "vector": "vector", "scalar": "scalar",
              "gpsimd": "gpsimd", "gdma0": "gpsimd", "gdma1": "gpsimd"}

# ---------------------------------------------------------------------------
# Custom DVE ops (runtime-registered into concourse.dve_ops.OPS).
import concourse.dve_ops as dve_ops_mod
from concourse.dve_ops import DveOp, OPS, CUSTOM_DVE_SPECS, _SUB_OPCODE_FOR_NAME, \
    _CUSTOM_DVE_ROW_BASE, TENSOR_TENSOR_REDUCE as OP_TTR
from concourse.dve_spec import (
    Spec, Src0, Src1, C0, C1, C2, Zero, One, Bin, AluOp, maxx, lower,
)
from concourse.dve_uop import DveOpSpec

RCP_C0 = -0.23549792     # Chebyshev seed scale (see RECIPROCAL_APPROX_FAST)
RCP_C1 = 2.0017324       # shared seed/NR constant


def _ref_sqsum(in0, in1, c0, c1, c2):
    b = (in0.astype(np.float32) ** 2 + in1.astype(np.float32) ** 2 + c0
         ).astype(np.float32)
    return b, b.reshape(b.shape[0], -1).sum(axis=-1, keepdims=True)


def _recip_fast1(x):
    # seed via exponent flip + one NR pass with c1 on both steps
    nx = (~x.astype(np.float32).view(np.int32)).view(np.float32)
    y0 = nx * np.float32(RCP_C0)
    return y0 * (np.float32(RCP_C1) - x * y0)


def _ref_recip2(in0, in1, c0, c1, c2):
    d = np.maximum(in0.astype(np.float32) + in1.astype(np.float32),
                   np.float32(c2))
    return _recip_fast1(d)


def _ref_recip1pt2(in0, in1, c0, c1, c2):
    d = in0.astype(np.float32) ** 2 + np.float32(1.0)
    b = _recip_fast1(d).astype(np.float32)
    return b, b.reshape(b.shape[0], -1).sum(axis=-1, keepdims=True)


def _ref_wt(in0, in1, c0, c1, c2):
    return (in0.astype(np.float32) * c0 + in1.astype(np.float32) * c1)


def _make_ops():
    # SQSUM: out = Src0^2 + Src1^2 + C0 ; accum = sum
    sqsum_body = Src0 * Src0 + Src1 * Src1 + C0
    sqsum = Spec(body=sqsum_body, accum=AluOp.ADD, accum_init=Zero,
                 reference=_ref_sqsum)

    # RECIP2: out = recip1(max(Src0+Src1, C2)) — 1/(uv+cr+eps)
    d = maxx(Src0 + Src1, C2)
    nx = Bin(AluOp.BITWISE_NOT, d, d)
    y0 = nx * C0
    recip2 = Spec(body=y0 * (C1 - d * y0), reference=_ref_recip2)

    # RECIP1PT2: out = recip1(1 + Src0^2); accum = sum  (corr: sum 1/(1+t^2))
    d2 = Src0 * Src0 + One
    nx2 = Bin(AluOp.BITWISE_NOT, d2, d2)
    y02 = nx2 * C0
    recip1pt2 = Spec(body=y02 * (C1 - d2 * y02), accum=AluOp.ADD,
                     accum_init=Zero, reference=_ref_recip1pt2)

    # WT: out = Src0*C0 + Src1*C1  (C0/C1 = per-partition scalar APs Sq, Sp)
    wt = Spec(body=Src0 * C0 + Src1 * C1, reference=_ref_wt)

    specs = {"ANT_CSI_SQSUM": sqsum, "ANT_CSI_RECIP2": recip2,
             "ANT_CSI_RECIP1PT2": recip1pt2, "ANT_CSI_WT": wt}
    ops = {}
    for name, spec in specs.items():
        if name in _SUB_OPCODE_FOR_NAME:
            ops[name] = next(o for o in OPS if o.name == name)
            continue
        row = _CUSTOM_DVE_ROW_BASE + len(OPS)
        sha = {}
        for ver in ("v3", "v4"):
            try:
                s = DveOpSpec(name=name, opcode=row, uops=lower(spec, ver=ver))
                sha[ver] = s.sha(ver)
            except Exception:
                pass
        op = DveOp(name, spec, subdim=False, uops_sha=sha)
        OPS.append(op)
        CUSTOM_DVE_SPECS[name] = spec
        _SUB_OPCODE_FOR_NAME[name] = row
        ops[name] = op
    return ops


_OPS = _make_ops()
OP_SQSUM = _OPS["ANT_CSI_SQSUM"]
OP_RECIP2 = _OPS["ANT_CSI_RECIP2"]
OP_RECIP1PT2 = _OPS["ANT_CSI_RECIP1PT2"]
OP_WT = _OPS["ANT_CSI_WT"]


# ---------------------------------------------------------------------------
class Sched:
    """Dependency scheduler for raw Bass (per-engine streams + semaphores)."""

    def __init__(self, nc):
        self.nc = nc
        self.ops = []
        self.cum = {e: 0 for e in _ENGINES}
        self.writer = {}
        self.readers = {}

    def add(self, engine, fn, reads=(), writes=(), inc=1):
        idx = len(self.ops)
        deps = set()
        for s in reads:
            w = self.writer.get(s)
            if w is not None:
                deps.add(w)
        for s in writes:
            for rd in self.readers.get(s, ()):
                deps.add(rd)
            w = self.writer.get(s)
            if w is not None:
                deps.add(w)
        self.cum[engine] += inc
        self.ops.append(dict(engine=engine, fn=fn, deps=deps, inc=inc,
                             cum=self.cum[engine], idx=idx))
        for s in reads:
            self.readers.setdefault(s, []).append(idx)
        for s in writes:
            self.writer[s] = idx
            self.readers[s] = []
        return idx

    def emit(self):
        nc = self.nc
        sems = {e: nc.alloc_semaphore(name=f"sem_{e}") for e in _ENGINES}
        streams = {s: [op for op in self.ops if _STREAM_OF[op["engine"]] == s]
                   for s in ("sync", "vector", "scalar", "gpsimd")}
        waited = {s: {p: 0 for p in _ENGINES}
                  for s in ("sync", "vector", "scalar", "gpsimd")}

        def run_stream(eng_handle, stream):
            for op in streams[stream]:
                need = {}
                for d in op["deps"]:
                    dop = self.ops[d]
                    pe = dop["engine"]
                    if _STREAM_OF[pe] == stream and pe == op["engine"]:
                        continue
                    need[pe] = max(need.get(pe, 0), dop["cum"])
                for pe, val in need.items():
                    if val > waited[stream][pe]:
                        eng_handle.wait_ge(sems[pe], val)
                        waited[stream][pe] = val
                inst = op["fn"]()
                inst.then_inc(sems[op["engine"]], op["inc"])

        with nc.Block() as block:
            @block.sync
            def _(sync):
                run_stream(sync, "sync")

            @block.vector
            def _(vector):
                run_stream(vector, "vector")

            @block.scalar
            def _(scalar):
                run_stream(scalar, "scalar")

            @block.gpsimd
            def _(gpsimd):
                run_stream(gpsimd, "gpsimd")

            total_s = self.cum["sync"]

            @block.gpsimd
            def _(gpsimd):
                gpsimd.wait_ge(sems["sync"], total_s)


# ---------------------------------------------------------------------------
def build_kernel(nblk=NBLK, reps=1):
    """reps>1 repeats the whole program (same data, same acc cols) for
    slope-based HW timing; results are identical to reps=1."""
    nc = bass.Bass(trn_type="TRN2")
    rows = nblk * 128

    # const AP for activation bias 0.0
    c0 = nc.alloc_sbuf_tensor("const-zero", [128, 1], F32)
    nc.gpsimd.memset(c0.ap(), 0.0)
    nc.const_aps.aps[(F32, 0.0)] = c0.ap()
    nc.all_engine_barrier()

    ins = {nm: nc.dram_tensor(nm, [rows, N], F32, kind="ExternalInput")
           for nm in ("pred_re", "pred_im", "target_re", "target_im")}
    nchunks = nblk * NCH
    acc_cols = nchunks * NSTAT
    acc_out = nc.dram_tensor("acc_out", [128, acc_cols], F32,
                             kind="ExternalOutput")

    def btile(nm, nbuf, w, dt=BF16):
        return [nc.alloc_sbuf_tensor(f"{nm}{i}", [128, w], dt).ap()
                for i in range(nbuf)]

    # block input tiles (bf16, full 4096 wide), double buffered
    a1B = btile("a1B", 2, N); b1B = btile("b1B", 2, N)
    a2B = btile("a2B", 2, N); b2B = btile("b2B", 2, N)
    # chunk tiles
    p2T = btile("p2", 2, CHUNK); q2T = btile("q2", 2, CHUNK)
    lnpT = btile("lnp", 2, CHUNK); lnqT = btile("lnq", 2, CHUNK)
    uT = btile("u", 2, CHUNK); vT = btile("v", 2, CHUNK)
    m1T = btile("m1", 1, CHUNK); m2T = btile("m2", 1, CHUNK)
    y1T = btile("y1", 1, CHUNK); y2T = btile("y2", 1, CHUNK)
    crT = btile("cr", 1, CHUNK); snT = btile("sn", 1, CHUNK)
    uvT = btile("uv", 1, CHUNK); idenT = btile("iden", 1, CHUNK)
    ratT = btile("rat", 2, CHUNK); ph2T = btile("ph2", 2, CHUNK)
    wtT = btile("wt", 2, CHUNK); lwT = btile("lw", 2, CHUNK)
    junkT = btile("junk", 1, CHUNK)

    acc = nc.alloc_sbuf_tensor("acc", [128, acc_cols], F32).ap()
    lil = nc.alloc_sbuf_tensor("lil", [128, 2 * nblk], F32).ap()

    sch = Sched(nc)

    def A(g, s):
        i = g * NSTAT + s
        return acc[:, i:i + 1], f"acc{i}"

    in_names = ("pred_re", "pred_im", "target_re", "target_im")
    in_tiles = (a1B, b1B, a2B, b2B)

    for gblk in range(nblk * reps):
        bkl = gblk % nblk
        ib = gblk % 2
        r0 = bkl * 128
        # casting loads (gpsimd SWDGE), one per tensor per block
        ld_idxs = []
        for nm, tl in zip(in_names, in_tiles):
            src = ins[nm][r0:r0 + 128, :]
            ld_idxs.append(sch.add(
                f"gdma{ib}",
                lambda d=tl[ib], s=src: nc.gpsimd.dma_start(d[:], s),
                writes=(f"{nm}B{ib}",), inc=16))
        # consumers must wait for the whole 4-load set (completions are
        # unordered within the set)
        for i in ld_idxs:
            sch.ops[i]["cum"] = sch.cum[f"gdma{ib}"]

        for c in range(NCH):
            g = bkl * NCH + c
            p = g % 2
            cs = slice(c * CHUNK, (c + 1) * CHUNK)
            a1, b1 = a1B[ib][:, cs], b1B[ib][:, cs]
            a2, b2 = a2B[ib][:, cs], b2B[ib][:, cs]
            rb = (f"pred_reB{ib}", f"pred_imB{ib}",
                  f"target_reB{ib}", f"target_imB{ib}")

            # --- DVE: squares with accums
            aap, asl = A(g, S_UU)
            sch.add("vector", lambda o=p2T[p], i0=a1, i1=b1, aa=aap:
                    nc.vector._custom_dve(OP_SQSUM, out=o[:], in0=i0, in1=i1,
                                          s0=1e-30, s1=0.0, accum_out=aa),
                    reads=(rb[0], rb[1]), writes=(f"p2{p}", asl))
            aap, asl = A(g, S_VV)
            sch.add("vector", lambda o=q2T[p], i0=a2, i1=b2, aa=aap:
                    nc.vector._custom_dve(OP_SQSUM, out=o[:], in0=i0, in1=i1,
                                          s0=1e-30, s1=0.0, accum_out=aa),
                    reads=(rb[2], rb[3]), writes=(f"q2{p}", asl))

            # --- Act: lnp, lnq, u, v  (natural_log_exp table)
            sch.add("scalar", lambda o=lnpT[p], i=p2T[p]:
                    nc.scalar.activation(o[:], i[:], AF.Ln),
                    reads=(f"p2{p}",), writes=(f"lnp{p}",))
            sch.add("scalar", lambda o=lnqT[p], i=q2T[p]:
                    nc.scalar.activation(o[:], i[:], AF.Ln),
                    reads=(f"q2{p}",), writes=(f"lnq{p}",))
            aap, asl = A(g, S_U)
            sch.add("scalar", lambda o=uT[p], i=lnpT[p], aa=aap:
                    nc.scalar.activation(o[:], i[:], AF.Exp, scale=0.5,
                                         accum_out=aa),
                    reads=(f"lnp{p}",), writes=(f"u{p}", asl))
            aap, asl = A(g, S_V)
            sch.add("scalar", lambda o=vT[p], i=lnqT[p], aa=aap:
                    nc.scalar.activation(o[:], i[:], AF.Exp, scale=0.5,
                                         accum_out=aa),
                    reads=(f"lnq{p}",), writes=(f"v{p}", asl))

            # --- cross products (bf16): m2/y2/cr/sn on Pool, m1/y1 on DVE
            sch.add("vector", lambda o=m1T[0], i0=a1, i1=a2:
                    nc.vector.tensor_tensor(out=o[:], in0=i0, in1=i1,
                                            op=ALU.mult),
                    reads=(rb[0], rb[2]), writes=("m1",))
            sch.add("gpsimd", lambda o=m2T[0], i0=b1, i1=b2:
                    nc.gpsimd.tensor_tensor(out=o[:], in0=i0, in1=i1,
                                            op=ALU.mult),
                    reads=(rb[1], rb[3]), writes=("m2",))
            sch.add("gpsimd", lambda o=crT[0], i0=m1T[0], i1=m2T[0]:
                    nc.gpsimd.tensor_tensor(out=o[:], in0=i0[:], in1=i1[:],
                                            op=ALU.add),
                    reads=("m1", "m2"), writes=("cr",))
            sch.add("vector", lambda o=y1T[0], i0=b1, i1=a2:
                    nc.vector.tensor_tensor(out=o[:], in0=i0, in1=i1,
                                            op=ALU.mult),
                    reads=(rb[1], rb[2]), writes=("y1",))
            sch.add("gpsimd", lambda o=y2T[0], i0=a1, i1=b2:
                    nc.gpsimd.tensor_tensor(out=o[:], in0=i0, in1=i1,
                                            op=ALU.mult),
                    reads=(rb[0], rb[3]), writes=("y2",))
            sch.add("gpsimd", lambda o=snT[0], i0=y1T[0], i1=y2T[0]:
                    nc.gpsimd.tensor_tensor(out=o[:], in0=i0[:], in1=i1[:],
                                            op=ALU.subtract),
                    reads=("y1", "y2"), writes=("sn",))

            # --- DVE: uv (+S_UV), iden, rat
            aap, asl = A(g, S_UV)
            sch.add("vector", lambda o=uvT[0], i0=uT[p], i1=vT[p], aa=aap:
                    nc.vector._custom_dve(OP_TTR, out=o[:], in0=i0[:],
                                          in1=i1[:], s0=0.0, s1=1.0,
                                          accum_out=aa),
                    reads=(f"u{p}", f"v{p}"), writes=("uv", asl))
            sch.add("vector", lambda o=idenT[0], i0=uvT[0], i1=crT[0]:
                    nc.vector._custom_dve(OP_RECIP2, out=o[:], in0=i0[:],
                                          in1=i1[:], s0=RCP_C0, s1=RCP_C1,
                                          imm2=1e-9),
                    reads=("uv", "cr"), writes=("iden",))
            sch.add("vector", lambda o=ratT[p], i0=snT[0], i1=idenT[0]:
                    nc.vector.tensor_tensor(out=o[:], in0=i0[:], in1=i1[:],
                                            op=ALU.mult),
                    reads=("sn", "iden"), writes=(f"rat{p}",))

            # --- Act: ph2 = Arctan(rat); S_PH += (2*ph2)^2  (both trig table)
            sch.add("scalar", lambda o=ph2T[p], i=ratT[p]:
                    nc.scalar.activation(o[:], i[:], AF.Arctan),
                    reads=(f"rat{p}",), writes=(f"ph2{p}",))
            aap, asl = A(g, S_PH)
            sch.add("scalar", lambda o=junkT[0], i=ph2T[p], aa=aap:
                    nc.scalar.activation(o[:], i[:], AF.Square, scale=2.0,
                                         accum_out=aa),
                    reads=(f"ph2{p}",), writes=("junks", asl))

            # --- DVE: corr accum
            aap, asl = A(g, S_IC)
            sch.add("vector", lambda o=junkT[0], i0=ratT[p], aa=aap:
                    nc.vector._custom_dve(OP_RECIP1PT2, out=o[:], in0=i0[:],
                                          s0=RCP_C0, s1=RCP_C1, accum_out=aa),
                    reads=(f"rat{p}",), writes=("junk", asl))

            # --- DVE: R8/R9 accums (custom TTR, c1=0.5 -> sum u ln u)
            aap, asl = A(g, S_R8)
            sch.add("vector", lambda o=junkT[0], i0=uT[p], i1=lnpT[p], aa=aap:
                    nc.vector._custom_dve(OP_TTR, out=o[:], in0=i0[:],
                                          in1=i1[:], s0=0.0, s1=0.5,
                                          accum_out=aa),
                    reads=(f"u{p}", f"lnp{p}"), writes=("junk", asl))
            aap, asl = A(g, S_R9)
            sch.add("vector", lambda o=junkT[0], i0=vT[p], i1=lnqT[p], aa=aap:
                    nc.vector._custom_dve(OP_TTR, out=o[:], in0=i0[:],
                                          in1=i1[:], s0=0.0, s1=0.5,
                                          accum_out=aa),
                    reads=(f"v{p}", f"lnq{p}"), writes=("junk", asl))

        # --- per-block: Sp = S_U(c0)+S_U(c1), Sq likewise. On Pool: the WT
        # custom op's scalar port reads lil at instruction ISSUE, so the
        # producer must be on another engine (cross-engine sem guarantees
        # the write is visible); a same-engine back-to-back write races.
        g0, g1 = bkl * NCH, bkl * NCH + 1
        lu = lil[:, 2 * bkl:2 * bkl + 1]
        lv = lil[:, 2 * bkl + 1:2 * bkl + 2]
        sch.add("gpsimd", lambda o=lu, i0=A(g0, S_U)[0], i1=A(g1, S_U)[0]:
                nc.gpsimd.tensor_tensor(out=o, in0=i0, in1=i1, op=ALU.add),
                reads=(A(g0, S_U)[1], A(g1, S_U)[1]), writes=(f"lu{bkl}",))
        sch.add("gpsimd", lambda o=lv, i0=A(g0, S_V)[0], i1=A(g1, S_V)[0]:
                nc.gpsimd.tensor_tensor(out=o, in0=i0, in1=i1, op=ALU.add),
                reads=(A(g0, S_V)[1], A(g1, S_V)[1]), writes=(f"lv{bkl}",))

        # --- wt pass per chunk: wt = Sq*u + Sp*v ; lw = Ln(wt); W accum
        for c in range(NCH):
            g = bkl * NCH + c
            p = g % 2
            sch.add("vector", lambda o=wtT[p], i0=uT[p], i1=vT[p],
                    s0=lv, s1=lu:
                    nc.vector._custom_dve(OP_WT, out=o[:], in0=i0[:],
                                          in1=i1[:], s0=s0, s1=s1),
                    reads=(f"u{p}", f"v{p}", f"lu{bkl}", f"lv{bkl}"),
                    writes=(f"wt{p}",))
            sch.add("scalar", lambda o=lwT[p], i=wtT[p]:
                    nc.scalar.activation(o[:], i[:], AF.Ln),
                    reads=(f"wt{p}",), writes=(f"lw{p}",))
            aap, asl = A(g, S_W)
            sch.add("vector", lambda o=junkT[0], i0=wtT[p], i1=lwT[p], aa=aap:
                    nc.vector._custom_dve(OP_TTR, out=o[:], in0=i0[:],
                                          in1=i1[:], s0=0.0, s1=1.0,
                                          accum_out=aa),
                    reads=(f"wt{p}", f"lw{p}"), writes=("junk", asl))

    all_acc = tuple(f"acc{i}" for i in range(acc_cols))
    sch.add("sync", lambda: nc.sync.dma_start(acc_out[:, :], acc[:, :]),
            reads=all_acc, writes=(), inc=16)

    sch.emit()
    # Bacc.compile()-equivalent finalization for the raw-Bass path:
    # gpsimd library loads (Pool SW ops execute garbage without them) and
    # typed-ISA instruction encoding (walrus rejects ISA len 0 otherwise).
    import bass_rust as _bass_rust
    from concourse.library_config import all_libraries, standard
    inst_type_to_lib_mask = {}
    for lib_i, lib in enumerate(all_libraries):
        for inst_type in lib.instructions:
            inst_type_to_lib_mask[inst_type] = (
                inst_type_to_lib_mask.get(inst_type, 0) | (1 << lib_i))
    _bass_rust.insert_library_loads(
        nc, inst_type_to_lib_mask, len(all_libraries), standard.index)
    mybir.codegen_inst_isa_subclasses(nc)
    return nc


# ---------------------------------------------------------------------------
def _host_reduce(accs, nblk=NBLK):
    """accs: list of per-core [128, nchunks*NSTAT] f32 -> final loss."""
    nchunks = nblk * NCH
    rows_per_core = nblk * 128
    stats = np.zeros((len(accs) * rows_per_core, NSTAT), np.float64)
    for k, a in enumerate(accs):
        a = a.astype(np.float64)
        for bkl in range(nblk):
            rows = slice(k * rows_per_core + bkl * 128,
                         k * rows_per_core + (bkl + 1) * 128)
            tot = np.zeros((128, NSTAT))
            for c in range(NCH):
                col0 = (bkl * NCH + c) * NSTAT
                tot += a[:, col0:col0 + NSTAT]
            stats[rows] = tot
    nrows = stats.shape[0]
    s_uu, s_vv = stats[:, S_UU], stats[:, S_VV]
    s_u, s_v, s_uv = stats[:, S_U], stats[:, S_V], stats[:, S_UV]
    s_ph, s_ic = stats[:, S_PH], stats[:, S_IC]
    r8, r9, W = stats[:, S_R8], stats[:, S_R9], stats[:, S_W]

    n = float(N)
    total = float(nrows) * n
    mag_loss = (s_uu - 2 * s_uv + s_vv).sum() / total
    p_mean, t_mean = s_u / n, s_v / n
    mean_loss = ((p_mean - t_mean) ** 2).mean()
    p_var = np.clip(s_uu / n - p_mean ** 2, 1e-12, None)
    t_var = np.clip(s_vv / n - t_mean ** 2, 1e-12, None)
    std_loss = ((np.sqrt(p_var) - np.sqrt(t_var)) ** 2).mean()
    # S_PH holds sum (2*ph2)^2 = sum dth^2 (Act Square with scale=2)
    phase_loss = s_ph.sum() / total
    cos_total = 2.0 * s_ic.sum() - total
    corr_loss = 2.0 - 2.0 * cos_total / total
    # r8/r9 are 0.5 * sum u*ln(p2) = sum u ln u (TTR scale=0.5)
    r8h, r9h = r8, r9
    js = 0.5 * (r8h / s_u + r9h / s_v - W / (s_u * s_v)
                + np.log(s_u) + np.log(s_v) + 2 * np.log(2.0))
    js_loss = js.mean()
    loss = (0.5 * mag_loss + 0.25 * mean_loss + 0.15 * std_loss
            + 0.5 * phase_loss + 0.2 * corr_loss + 0.1 * js_loss)
    return loss


_NC_CACHE = None


def _get_nc():
    global _NC_CACHE
    if _NC_CACHE is None:
        _NC_CACHE = build_kernel()
    return _NC_CACHE


def kernel(pred_re, pred_im, target_re, target_im, _trace=False):
    nc = _get_nc()
    arrs = {"pred_re": pred_re, "pred_im": pred_im,
            "target_re": target_re, "target_im": target_im}
    in_maps = []
    for k in range(NCORES):
        rows = slice(k * ROWS_PER_CORE, (k + 1) * ROWS_PER_CORE)
        in_maps.append({nm: np.ascontiguousarray(
            np.asarray(a)[rows], dtype=np.float32) for nm, a in arrs.items()})
    res = run_bass_kernel_spmd(nc, in_maps, core_ids=list(range(NCORES)),
                               trace=_trace)
    accs = [res.results[k]["acc_out"] for k in range(NCORES)]
    loss = _host_reduce(accs)
    out = np.float32(loss)
    if _trace:
        return out, res
    return out


# revision 7
# speedup vs baseline: 1.0848x; 1.0848x over previous
"""CSI loss kernel v2 for Trainium2 (8 NeuronCores, data parallel).

Math (see reference.py; eps terms dropped where negligible for randn):
  u = |p|, v = |t|.  p2 = a1^2+b1^2 (+1e-30), q2 = a2^2+b2^2 (+1e-30)
  lnp = ln(p2) = 2 ln u;  u = exp(0.5 lnp)   (keeps Act in ONE table)
  cross: cr = a1a2+b1b2, sn = b1a2-a1b2  (p * conj(t) = cr + i sn)
  phase: dth = 2*atan(sn / (uv + cr + eps))   (half-angle, no fixup)
  corr:  cos(dth) = 2/(1+rat^2) - 1  ->  S_IC = sum 1/(1+rat^2)
  js:    R8 = sum u*lnp (=2 sum u ln u), W = sum wt ln wt,
         wt = Sq*u + Sp*v with per-row Sp = sum u, Sq = sum v.

Engine split per chunk [128,2048]:
  Pool: 4 casting DMA loads (f32->bf16, block-batched), phacc/R8/R9 accums
  DVE:  SQSUM/RECIP2/CORR/WT custom ops, TTR accums, bf16 products
  Act:  Ln/Exp (natural_log_exp table) + Arctan (trig table) + Ln(wt)
"""

import numpy as np

import concourse.bass as bass
import concourse.mybir as mybir
from concourse.bass_utils import run_bass_kernel_spmd

AF = mybir.ActivationFunctionType
ALU = mybir.AluOpType
F32 = mybir.dt.float32
BF16 = mybir.dt.bfloat16

B, N = 4096, 4096
NCORES = 8
ROWS_PER_CORE = B // NCORES          # 512
NBLK = ROWS_PER_CORE // 128          # 4 row-blocks of 128
CHUNK = 2048
NCH = N // CHUNK                     # 2 col-chunks per block
NSTAT = 10
S_UU, S_VV, S_U, S_V, S_UV, S_PH, S_IC, S_R8, S_R9, S_W = range(NSTAT)
NCHUNKS = NBLK * NCH                 # 8
ACC_COLS = NCHUNKS * NSTAT           # 80

_ENGINES = ("sync", "vector", "scalar", "gpsimd", "gdma0", "gdma1")
# "gdmaN": DMA ops issued on the gpsimd engine stream but tracked on a
# per-buffer-parity semaphore (DMA completions are async and unordered;
# all waits land on whole-block-set totals).
_STREAM_OF = {"sync": "sync", "vector": "vector", "scalar": "scalar",
              "gpsimd": "gpsimd", "gdma0": "gpsimd", "gdma1": "gpsimd"}

# ---------------------------------------------------------------------------
# Custom DVE ops (runtime-registered into concourse.dve_ops.OPS).
import concourse.dve_ops as dve_ops_mod
from concourse.dve_ops import DveOp, OPS, CUSTOM_DVE_SPECS, _SUB_OPCODE_FOR_NAME, \
    _CUSTOM_DVE_ROW_BASE, TENSOR_TENSOR_REDUCE as OP_TTR
from concourse.dve_spec import (
    Spec, Src0, Src1, C0, C1, C2, Zero, One, Bin, AluOp, maxx, lower,
)
from concourse.dve_uop import DveOpSpec

RCP_C0 = -0.23549792     # Chebyshev seed scale (see RECIPROCAL_APPROX_FAST)
RCP_C1 = 2.0017324       # shared seed/NR constant


def _ref_sqsum(in0, in1, c0, c1, c2):
    b = (in0.astype(np.float32) ** 2 + in1.astype(np.float32) ** 2 + c0
         ).astype(np.float32)
    return b, b.reshape(b.shape[0], -1).sum(axis=-1, keepdims=True)


def _recip_fast1(x):
    # seed via exponent flip + one NR pass with c1 on both steps
    nx = (~x.astype(np.float32).view(np.int32)).view(np.float32)
    y0 = nx * np.float32(RCP_C0)
    return y0 * (np.float32(RCP_C1) - x * y0)


def _ref_recip2(in0, in1, c0, c1, c2):
    d = np.maximum(in0.astype(np.float32) + in1.astype(np.float32),
                   np.float32(c2))
    return _recip_fast1(d)


def _ref_recip1pt2(in0, in1, c0, c1, c2):
    d = in0.astype(np.float32) ** 2 + np.float32(1.0)
    b = _recip_fast1(d).astype(np.float32)
    return b, b.reshape(b.shape[0], -1).sum(axis=-1, keepdims=True)


def _ref_wt(in0, in1, c0, c1, c2):
    return (in0.astype(np.float32) * c0 + in1.astype(np.float32) * c1)


def _make_ops():
    # SQSUM: out = Src0^2 + Src1^2 + C0 ; accum = sum
    sqsum_body = Src0 * Src0 + Src1 * Src1 + C0
    sqsum = Spec(body=sqsum_body, accum=AluOp.ADD, accum_init=Zero,
                 reference=_ref_sqsum)

    # RECIP2: out = recip1(max(Src0+Src1, C2)) — 1/(uv+cr+eps)
    d = maxx(Src0 + Src1, C2)
    nx = Bin(AluOp.BITWISE_NOT, d, d)
    y0 = nx * C0
    recip2 = Spec(body=y0 * (C1 - d * y0), reference=_ref_recip2)

    # RECIP1PT2: out = recip1(1 + Src0^2); accum = sum  (corr: sum 1/(1+t^2))
    d2 = Src0 * Src0 + One
    nx2 = Bin(AluOp.BITWISE_NOT, d2, d2)
    y02 = nx2 * C0
    recip1pt2 = Spec(body=y02 * (C1 - d2 * y02), accum=AluOp.ADD,
                     accum_init=Zero, reference=_ref_recip1pt2)

    # WT: out = Src0*C0 + Src1*C1  (C0/C1 = per-partition scalar APs Sq, Sp)
    wt = Spec(body=Src0 * C0 + Src1 * C1, reference=_ref_wt)

    specs = {"ANT_CSI_SQSUM": sqsum, "ANT_CSI_RECIP2": recip2,
             "ANT_CSI_RECIP1PT2": recip1pt2, "ANT_CSI_WT": wt}
    ops = {}
    for name, spec in specs.items():
        if name in _SUB_OPCODE_FOR_NAME:
            ops[name] = next(o for o in OPS if o.name == name)
            continue
        row = _CUSTOM_DVE_ROW_BASE + len(OPS)
        sha = {}
        for ver in ("v3", "v4"):
            try:
                s = DveOpSpec(name=name, opcode=row, uops=lower(spec, ver=ver))
                sha[ver] = s.sha(ver)
            except Exception:
                pass
        op = DveOp(name, spec, subdim=False, uops_sha=sha)
        OPS.append(op)
        CUSTOM_DVE_SPECS[name] = spec
        _SUB_OPCODE_FOR_NAME[name] = row
        ops[name] = op
    return ops


_OPS = _make_ops()
OP_SQSUM = _OPS["ANT_CSI_SQSUM"]
OP_RECIP2 = _OPS["ANT_CSI_RECIP2"]
OP_RECIP1PT2 = _OPS["ANT_CSI_RECIP1PT2"]
OP_WT = _OPS["ANT_CSI_WT"]


# ---------------------------------------------------------------------------
class Sched:
    """Dependency scheduler for raw Bass (per-engine streams + semaphores)."""

    def __init__(self, nc):
        self.nc = nc
        self.ops = []
        self.cum = {e: 0 for e in _ENGINES}
        self.writer = {}
        self.readers = {}

    def add(self, engine, fn, reads=(), writes=(), inc=1):
        idx = len(self.ops)
        deps = set()
        for s in reads:
            w = self.writer.get(s)
            if w is not None:
                deps.add(w)
        for s in writes:
            for rd in self.readers.get(s, ()):
                deps.add(rd)
            w = self.writer.get(s)
            if w is not None:
                deps.add(w)
        self.cum[engine] += inc
        self.ops.append(dict(engine=engine, fn=fn, deps=deps, inc=inc,
                             cum=self.cum[engine], idx=idx))
        for s in reads:
            self.readers.setdefault(s, []).append(idx)
        for s in writes:
            self.writer[s] = idx
            self.readers[s] = []
        return idx

    def emit(self):
        nc = self.nc
        sems = {e: nc.alloc_semaphore(name=f"sem_{e}") for e in _ENGINES}
        streams = {s: [op for op in self.ops if _STREAM_OF[op["engine"]] == s]
                   for s in ("sync", "vector", "scalar", "gpsimd")}
        waited = {s: {p: 0 for p in _ENGINES}
                  for s in ("sync", "vector", "scalar", "gpsimd")}

        def run_stream(eng_handle, stream):
            for op in streams[stream]:
                need = {}
                for d in op["deps"]:
                    dop = self.ops[d]
                    pe = dop["engine"]
                    if _STREAM_OF[pe] == stream and pe == op["engine"]:
                        continue
                    need[pe] = max(need.get(pe, 0), dop["cum"])
                for pe, val in need.items():
                    if val > waited[stream][pe]:
                        eng_handle.wait_ge(sems[pe], val)
                        waited[stream][pe] = val
                inst = op["fn"]()
                inst.then_inc(sems[op["engine"]], op["inc"])

        with nc.Block() as block:
            @block.sync
            def _(sync):
                run_stream(sync, "sync")

            @block.vector
            def _(vector):
                run_stream(vector, "vector")

            @block.scalar
            def _(scalar):
                run_stream(scalar, "scalar")

            @block.gpsimd
            def _(gpsimd):
                run_stream(gpsimd, "gpsimd")

            total_s = self.cum["sync"]

            @block.gpsimd
            def _(gpsimd):
                gpsimd.wait_ge(sems["sync"], total_s)


# ---------------------------------------------------------------------------
def build_kernel(nblk=NBLK, reps=1):
    """reps>1 repeats the whole program (same data, same acc cols) for
    slope-based HW timing; results are identical to reps=1."""
    nc = bass.Bass(trn_type="TRN2")
    rows = nblk * 128

    # const AP for activation bias 0.0
    c0 = nc.alloc_sbuf_tensor("const-zero", [128, 1], F32)
    nc.gpsimd.memset(c0.ap(), 0.0)
    nc.const_aps.aps[(F32, 0.0)] = c0.ap()
    nc.all_engine_barrier()

    ins = {nm: nc.dram_tensor(nm, [rows, N], F32, kind="ExternalInput")
           for nm in ("pred_re", "pred_im", "target_re", "target_im")}
    nchunks = nblk * NCH
    acc_cols = nchunks * NSTAT
    acc_out = nc.dram_tensor("acc_out", [128, acc_cols], F32,
                             kind="ExternalOutput")

    def btile(nm, nbuf, w, dt=BF16):
        return [nc.alloc_sbuf_tensor(f"{nm}{i}", [128, w], dt).ap()
                for i in range(nbuf)]

    # block input tiles (bf16, full 4096 wide), double buffered
    a1B = btile("a1B", 2, N); b1B = btile("b1B", 2, N)
    a2B = btile("a2B", 2, N); b2B = btile("b2B", 2, N)
    # chunk tiles
    p2T = btile("p2", 2, CHUNK); q2T = btile("q2", 2, CHUNK)
    lnpT = btile("lnp", 2, CHUNK); lnqT = btile("lnq", 2, CHUNK)
    uT = btile("u", 2, CHUNK); vT = btile("v", 2, CHUNK)
    m1T = btile("m1", 1, CHUNK); m2T = btile("m2", 1, CHUNK)
    y1T = btile("y1", 1, CHUNK); y2T = btile("y2", 1, CHUNK)
    crT = btile("cr", 1, CHUNK); snT = btile("sn", 1, CHUNK)
    uvT = btile("uv", 1, CHUNK); idenT = btile("iden", 1, CHUNK)
    ratT = btile("rat", 2, CHUNK); ph2T = btile("ph2", 2, CHUNK)
    wtT = btile("wt", 4, CHUNK); lwT = btile("lw", 4, CHUNK)
    junkT = btile("junk", 1, CHUNK)

    acc = nc.alloc_sbuf_tensor("acc", [128, acc_cols], F32).ap()
    lil = nc.alloc_sbuf_tensor("lil", [128, 2 * nblk], F32).ap()

    sch = Sched(nc)

    def A(g, s):
        i = g * NSTAT + s
        return acc[:, i:i + 1], f"acc{i}"

    in_names = ("pred_re", "pred_im", "target_re", "target_im")
    in_tiles = (a1B, b1B, a2B, b2B)

    # Deferred wt-tail ops (lw + Wacc) from the previous block, flushed
    # inside the next block's ln/exp Act section so the Act stream is
    # [lnexp x10][trig x4] per block: 2 table loads instead of 4.
    pending_tail = []

    for gblk in range(nblk * reps):
        bkl = gblk % nblk
        ib = gblk % 2
        bq = (gblk % 2) * 2  # wt/lw buffer base for this block
        r0 = bkl * 128
        # casting loads (gpsimd SWDGE), one per tensor per block
        ld_idxs = []
        for nm, tl in zip(in_names, in_tiles):
            src = ins[nm][r0:r0 + 128, :]
            ld_idxs.append(sch.add(
                f"gdma{ib}",
                lambda d=tl[ib], s=src: nc.gpsimd.dma_start(d[:], s),
                writes=(f"{nm}B{ib}",), inc=16))
        # consumers must wait for the whole 4-load set (completions are
        # unordered within the set)
        for i in ld_idxs:
            sch.ops[i]["cum"] = sch.cum[f"gdma{ib}"]

        rb = (f"pred_reB{ib}", f"pred_imB{ib}",
              f"target_reB{ib}", f"target_imB{ib}")

        def chunk_aps(c):
            cs = slice(c * CHUNK, (c + 1) * CHUNK)
            return (a1B[ib][:, cs], b1B[ib][:, cs],
                    a2B[ib][:, cs], b2B[ib][:, cs])

        # --- phase 1 (per chunk): squares + ln/exp (Act: natural_log_exp)
        for c in range(NCH):
            g = bkl * NCH + c
            p = g % 2
            a1, b1, a2, b2 = chunk_aps(c)
            aap, asl = A(g, S_UU)
            sch.add("vector", lambda o=p2T[p], i0=a1, i1=b1, aa=aap:
                    nc.vector._custom_dve(OP_SQSUM, out=o[:], in0=i0, in1=i1,
                                          s0=1e-30, s1=0.0, accum_out=aa),
                    reads=(rb[0], rb[1]), writes=(f"p2{p}", asl))
            aap, asl = A(g, S_VV)
            sch.add("vector", lambda o=q2T[p], i0=a2, i1=b2, aa=aap:
                    nc.vector._custom_dve(OP_SQSUM, out=o[:], in0=i0, in1=i1,
                                          s0=1e-30, s1=0.0, accum_out=aa),
                    reads=(rb[2], rb[3]), writes=(f"q2{p}", asl))
            sch.add("scalar", lambda o=lnpT[p], i=p2T[p]:
                    nc.scalar.activation(o[:], i[:], AF.Ln),
                    reads=(f"p2{p}",), writes=(f"lnp{p}",))
            sch.add("scalar", lambda o=lnqT[p], i=q2T[p]:
                    nc.scalar.activation(o[:], i[:], AF.Ln),
                    reads=(f"q2{p}",), writes=(f"lnq{p}",))
            aap, asl = A(g, S_U)
            sch.add("scalar", lambda o=uT[p], i=lnpT[p], aa=aap:
                    nc.scalar.activation(o[:], i[:], AF.Exp, scale=0.5,
                                         accum_out=aa),
                    reads=(f"lnp{p}",), writes=(f"u{p}", asl))
            aap, asl = A(g, S_V)
            sch.add("scalar", lambda o=vT[p], i=lnqT[p], aa=aap:
                    nc.scalar.activation(o[:], i[:], AF.Exp, scale=0.5,
                                         accum_out=aa),
                    reads=(f"lnq{p}",), writes=(f"v{p}", asl))

        # --- flush previous block's wt tail (lw is ln table: no switch)
        for fn in pending_tail:
            fn()
        pending_tail = []

        # --- phase 2 (per chunk): cross products, phase ratio, arctan
        for c in range(NCH):
            g = bkl * NCH + c
            p = g % 2
            a1, b1, a2, b2 = chunk_aps(c)
            sch.add("vector", lambda o=m1T[0], i0=a1, i1=a2:
                    nc.vector.tensor_tensor(out=o[:], in0=i0, in1=i1,
                                            op=ALU.mult),
                    reads=(rb[0], rb[2]), writes=("m1",))
            sch.add("gpsimd", lambda o=m2T[0], i0=b1, i1=b2:
                    nc.gpsimd.tensor_tensor(out=o[:], in0=i0, in1=i1,
                                            op=ALU.mult),
                    reads=(rb[1], rb[3]), writes=("m2",))
            sch.add("gpsimd", lambda o=crT[0], i0=m1T[0], i1=m2T[0]:
                    nc.gpsimd.tensor_tensor(out=o[:], in0=i0[:], in1=i1[:],
                                            op=ALU.add),
                    reads=("m1", "m2"), writes=("cr",))
            sch.add("vector", lambda o=y1T[0], i0=b1, i1=a2:
                    nc.vector.tensor_tensor(out=o[:], in0=i0, in1=i1,
                                            op=ALU.mult),
                    reads=(rb[1], rb[2]), writes=("y1",))
            sch.add("gpsimd", lambda o=y2T[0], i0=a1, i1=b2:
                    nc.gpsimd.tensor_tensor(out=o[:], in0=i0, in1=i1,
                                            op=ALU.mult),
                    reads=(rb[0], rb[3]), writes=("y2",))
            sch.add("gpsimd", lambda o=snT[0], i0=y1T[0], i1=y2T[0]:
                    nc.gpsimd.tensor_tensor(out=o[:], in0=i0[:], in1=i1[:],
                                            op=ALU.subtract),
                    reads=("y1", "y2"), writes=("sn",))

            aap, asl = A(g, S_UV)
            sch.add("vector", lambda o=uvT[0], i0=uT[p], i1=vT[p], aa=aap:
                    nc.vector._custom_dve(OP_TTR, out=o[:], in0=i0[:],
                                          in1=i1[:], s0=0.0, s1=1.0,
                                          accum_out=aa),
                    reads=(f"u{p}", f"v{p}"), writes=("uv", asl))
            sch.add("vector", lambda o=idenT[0], i0=uvT[0], i1=crT[0]:
                    nc.vector._custom_dve(OP_RECIP2, out=o[:], in0=i0[:],
                                          in1=i1[:], s0=RCP_C0, s1=RCP_C1,
                                          imm2=1e-9),
                    reads=("uv", "cr"), writes=("iden",))
            sch.add("vector", lambda o=ratT[p], i0=snT[0], i1=idenT[0]:
                    nc.vector.tensor_tensor(out=o[:], in0=i0[:], in1=i1[:],
                                            op=ALU.mult),
                    reads=("sn", "iden"), writes=(f"rat{p}",))

            # Act trig section: ph2 = Arctan(rat); S_PH += (2*ph2)^2
            sch.add("scalar", lambda o=ph2T[p], i=ratT[p]:
                    nc.scalar.activation(o[:], i[:], AF.Arctan),
                    reads=(f"rat{p}",), writes=(f"ph2{p}",))
            aap, asl = A(g, S_PH)
            sch.add("scalar", lambda o=junkT[0], i=ph2T[p], aa=aap:
                    nc.scalar.activation(o[:], i[:], AF.Square, scale=2.0,
                                         accum_out=aa),
                    reads=(f"ph2{p}",), writes=("junks", asl))

            aap, asl = A(g, S_IC)
            sch.add("vector", lambda o=junkT[0], i0=ratT[p], aa=aap:
                    nc.vector._custom_dve(OP_RECIP1PT2, out=o[:], in0=i0[:],
                                          s0=RCP_C0, s1=RCP_C1, accum_out=aa),
                    reads=(f"rat{p}",), writes=("junk", asl))

            aap, asl = A(g, S_R8)
            sch.add("vector", lambda o=junkT[0], i0=uT[p], i1=lnpT[p], aa=aap:
                    nc.vector._custom_dve(OP_TTR, out=o[:], in0=i0[:],
                                          in1=i1[:], s0=0.0, s1=0.5,
                                          accum_out=aa),
                    reads=(f"u{p}", f"lnp{p}"), writes=("junk", asl))
            aap, asl = A(g, S_R9)
            sch.add("vector", lambda o=junkT[0], i0=vT[p], i1=lnqT[p], aa=aap:
                    nc.vector._custom_dve(OP_TTR, out=o[:], in0=i0[:],
                                          in1=i1[:], s0=0.0, s1=0.5,
                                          accum_out=aa),
                    reads=(f"v{p}", f"lnq{p}"), writes=("junk", asl))

        # --- per-block: Sp = S_U(c0)+S_U(c1), Sq likewise. On Pool: the WT
        # custom op's scalar port reads lil at instruction ISSUE, so the
        # producer must be on another engine (cross-engine sem guarantees
        # the write is visible); a same-engine back-to-back write races.
        g0, g1 = bkl * NCH, bkl * NCH + 1
        lu = lil[:, 2 * bkl:2 * bkl + 1]
        lv = lil[:, 2 * bkl + 1:2 * bkl + 2]
        sch.add("gpsimd", lambda o=lu, i0=A(g0, S_U)[0], i1=A(g1, S_U)[0]:
                nc.gpsimd.tensor_tensor(out=o, in0=i0, in1=i1, op=ALU.add),
                reads=(A(g0, S_U)[1], A(g1, S_U)[1]), writes=(f"lu{bkl}",))
        sch.add("gpsimd", lambda o=lv, i0=A(g0, S_V)[0], i1=A(g1, S_V)[0]:
                nc.gpsimd.tensor_tensor(out=o, in0=i0, in1=i1, op=ALU.add),
                reads=(A(g0, S_V)[1], A(g1, S_V)[1]), writes=(f"lv{bkl}",))

        # --- wt = Sq*u + Sp*v now; Ln(wt) + W accum deferred to next block
        for c in range(NCH):
            g = bkl * NCH + c
            p = g % 2
            q = bq + c
            sch.add("vector", lambda o=wtT[q], i0=uT[p], i1=vT[p],
                    s0=lv, s1=lu:
                    nc.vector._custom_dve(OP_WT, out=o[:], in0=i0[:],
                                          in1=i1[:], s0=s0, s1=s1),
                    reads=(f"u{p}", f"v{p}", f"lu{bkl}", f"lv{bkl}"),
                    writes=(f"wt{q}",))

            def tail(g=g, p=p, q=q):
                sch.add("scalar", lambda o=lwT[q], i=wtT[q]:
                        nc.scalar.activation(o[:], i[:], AF.Ln),
                        reads=(f"wt{q}",), writes=(f"lw{q}",))
                aap, asl = A(g, S_W)
                sch.add("vector", lambda o=junkT[0], i0=wtT[q], i1=lwT[q],
                        aa=aap:
                        nc.vector._custom_dve(OP_TTR, out=o[:], in0=i0[:],
                                              in1=i1[:], s0=0.0, s1=1.0,
                                              accum_out=aa),
                        reads=(f"wt{q}", f"lw{q}"), writes=("junk", asl))

            pending_tail.append(tail)

    for fn in pending_tail:
        fn()
    pending_tail = []

    all_acc = tuple(f"acc{i}" for i in range(acc_cols))
    sch.add("sync", lambda: nc.sync.dma_start(acc_out[:, :], acc[:, :]),
            reads=all_acc, writes=(), inc=16)

    sch.emit()
    # Bacc.compile()-equivalent finalization for the raw-Bass path:
    # gpsimd library loads (Pool SW ops execute garbage without them) and
    # typed-ISA instruction encoding (walrus rejects ISA len 0 otherwise).
    import bass_rust as _bass_rust
    from concourse.library_config import all_libraries, standard
    inst_type_to_lib_mask = {}
    for lib_i, lib in enumerate(all_libraries):
        for inst_type in lib.instructions:
            inst_type_to_lib_mask[inst_type] = (
                inst_type_to_lib_mask.get(inst_type, 0) | (1 << lib_i))
    _bass_rust.insert_library_loads(
        nc, inst_type_to_lib_mask, len(all_libraries), standard.index)
    mybir.codegen_inst_isa_subclasses(nc)
    return nc


# ---------------------------------------------------------------------------
def _host_reduce(accs, nblk=NBLK):
    """accs: list of per-core [128, nchunks*NSTAT] f32 -> final loss."""
    nchunks = nblk * NCH
    rows_per_core = nblk * 128
    stats = np.zeros((len(accs) * rows_per_core, NSTAT), np.float64)
    for k, a in enumerate(accs):
        a = a.astype(np.float64)
        for bkl in range(nblk):
            rows = slice(k * rows_per_core + bkl * 128,
                         k * rows_per_core + (bkl + 1) * 128)
            tot = np.zeros((128, NSTAT))
            for c in range(NCH):
                col0 = (bkl * NCH + c) * NSTAT
                tot += a[:, col0:col0 + NSTAT]
            stats[rows] = tot
    nrows = stats.shape[0]
    s_uu, s_vv = stats[:, S_UU], stats[:, S_VV]
    s_u, s_v, s_uv = stats[:, S_U], stats[:, S_V], stats[:, S_UV]
    s_ph, s_ic = stats[:, S_PH], stats[:, S_IC]
    r8, r9, W = stats[:, S_R8], stats[:, S_R9], stats[:, S_W]

    n = float(N)
    total = float(nrows) * n
    mag_loss = (s_uu - 2 * s_uv + s_vv).sum() / total
    p_mean, t_mean = s_u / n, s_v / n
    mean_loss = ((p_mean - t_mean) ** 2).mean()
    p_var = np.clip(s_uu / n - p_mean ** 2, 1e-12, None)
    t_var = np.clip(s_vv / n - t_mean ** 2, 1e-12, None)
    std_loss = ((np.sqrt(p_var) - np.sqrt(t_var)) ** 2).mean()
    # S_PH holds sum (2*ph2)^2 = sum dth^2 (Act Square with scale=2)
    phase_loss = s_ph.sum() / total
    cos_total = 2.0 * s_ic.sum() - total
    corr_loss = 2.0 - 2.0 * cos_total / total
    # r8/r9 are 0.5 * sum u*ln(p2) = sum u ln u (TTR scale=0.5)
    r8h, r9h = r8, r9
    js = 0.5 * (r8h / s_u + r9h / s_v - W / (s_u * s_v)
                + np.log(s_u) + np.log(s_v) + 2 * np.log(2.0))
    js_loss = js.mean()
    loss = (0.5 * mag_loss + 0.25 * mean_loss + 0.15 * std_loss
            + 0.5 * phase_loss + 0.2 * corr_loss + 0.1 * js_loss)
    return loss


_NC_CACHE = None


def _get_nc():
    global _NC_CACHE
    if _NC_CACHE is None:
        _NC_CACHE = build_kernel()
    return _NC_CACHE


def kernel(pred_re, pred_im, target_re, target_im, _trace=False):
    nc = _get_nc()
    arrs = {"pred_re": pred_re, "pred_im": pred_im,
            "target_re": target_re, "target_im": target_im}
    in_maps = []
    for k in range(NCORES):
        rows = slice(k * ROWS_PER_CORE, (k + 1) * ROWS_PER_CORE)
        in_maps.append({nm: np.ascontiguousarray(
            np.asarray(a)[rows], dtype=np.float32) for nm, a in arrs.items()})
    res = run_bass_kernel_spmd(nc, in_maps, core_ids=list(range(NCORES)),
                               trace=_trace)
    accs = [res.results[k]["acc_out"] for k in range(NCORES)]
    loss = _host_reduce(accs)
    out = np.float32(loss)
    if _trace:
        return out, res
    return out


# revision 8
# speedup vs baseline: 2.2136x; 2.0406x over previous
"""CSI loss kernel v2 for Trainium2 (8 NeuronCores, data parallel).

Math (see reference.py; eps terms dropped where negligible for randn):
  u = |p|, v = |t|.  p2 = a1^2+b1^2 (+1e-30), q2 = a2^2+b2^2 (+1e-30)
  lnp = ln(p2) = 2 ln u;  u = exp(0.5 lnp)   (keeps Act in ONE table)
  cross: cr = a1a2+b1b2, sn = b1a2-a1b2  (p * conj(t) = cr + i sn)
  phase: dth = 2*atan(sn / (uv + cr + eps))   (half-angle, no fixup)
  corr:  cos(dth) = 2/(1+rat^2) - 1  ->  S_IC = sum 1/(1+rat^2)
  js:    R8 = sum u*lnp (=2 sum u ln u), W = sum wt ln wt,
         wt = Sq*u + Sp*v with per-row Sp = sum u, Sq = sum v.

Engine split per chunk [128,2048]:
  Pool: 4 casting DMA loads (f32->bf16, block-batched), phacc/R8/R9 accums
  DVE:  SQSUM/RECIP2/CORR/WT custom ops, TTR accums, bf16 products
  Act:  Ln/Exp (natural_log_exp table) + Arctan (trig table) + Ln(wt)
"""

import numpy as np

import concourse.bass as bass
import concourse.mybir as mybir
from concourse.bass_utils import run_bass_kernel_spmd

AF = mybir.ActivationFunctionType
ALU = mybir.AluOpType
F32 = mybir.dt.float32
BF16 = mybir.dt.bfloat16

B, N = 4096, 4096
NCORES = 8
ROWS_PER_CORE = B // NCORES          # 512
NBLK = ROWS_PER_CORE // 128          # 4 row-blocks of 128
CHUNK = 2048
NCH = N // CHUNK                     # 2 col-chunks per block
NSTAT = 10
S_UU, S_VV, S_U, S_V, S_UV, S_PH, S_IC, S_R8, S_R9, S_W = range(NSTAT)
NCHUNKS = NBLK * NCH                 # 8
ACC_COLS = NCHUNKS * NSTAT           # 80

_ENGINES = ("sync", "vector", "scalar", "gpsimd", "gdma0", "gdma1")
# "gdmaN": DMA ops issued on the gpsimd engine stream but tracked on a
# per-buffer-parity semaphore (DMA completions are async and unordered;
# all waits land on whole-block-set totals).
_STREAM_OF = {"sync": "sync", "vector": "vector", "scalar": "scalar",
              "gpsimd": "gpsimd", "gdma0": "gpsimd", "gdma1": "gpsimd"}

# ---------------------------------------------------------------------------
# Custom DVE ops (runtime-registered into concourse.dve_ops.OPS).
import concourse.dve_ops as dve_ops_mod
from concourse.dve_ops import DveOp, OPS, CUSTOM_DVE_SPECS, _SUB_OPCODE_FOR_NAME, \
    _CUSTOM_DVE_ROW_BASE, TENSOR_TENSOR_REDUCE as OP_TTR
from concourse.dve_spec import (
    Spec, Src0, Src1, C0, C1, C2, Zero, One, Bin, AluOp, maxx, lower,
)
from concourse.dve_uop import DveOpSpec

RCP_C0 = -0.23549792     # Chebyshev seed scale (see RECIPROCAL_APPROX_FAST)
RCP_C1 = 2.0017324       # shared seed/NR constant


def _ref_sqsum(in0, in1, c0, c1, c2):
    b = (in0.astype(np.float32) ** 2 + in1.astype(np.float32) ** 2 + c0
         ).astype(np.float32)
    return b, b.reshape(b.shape[0], -1).sum(axis=-1, keepdims=True)


def _recip_fast1(x):
    # seed via exponent flip + one NR pass with c1 on both steps
    nx = (~x.astype(np.float32).view(np.int32)).view(np.float32)
    y0 = nx * np.float32(RCP_C0)
    return y0 * (np.float32(RCP_C1) - x * y0)


def _ref_recip2(in0, in1, c0, c1, c2):
    d = np.maximum(in0.astype(np.float32) + in1.astype(np.float32),
                   np.float32(c2))
    return _recip_fast1(d)


def _ref_recip1pt2(in0, in1, c0, c1, c2):
    d = in0.astype(np.float32) ** 2 + np.float32(1.0)
    b = _recip_fast1(d).astype(np.float32)
    return b, b.reshape(b.shape[0], -1).sum(axis=-1, keepdims=True)


def _ref_wt(in0, in1, c0, c1, c2):
    return (in0.astype(np.float32) * c0 + in1.astype(np.float32) * c1)


def _make_ops():
    # SQSUM: out = Src0^2 + Src1^2 + C0 ; accum = sum
    sqsum_body = Src0 * Src0 + Src1 * Src1 + C0
    sqsum = Spec(body=sqsum_body, accum=AluOp.ADD, accum_init=Zero,
                 reference=_ref_sqsum)

    # RECIP2: out = recip1(max(Src0+Src1, C2)) — 1/(uv+cr+eps)
    d = maxx(Src0 + Src1, C2)
    nx = Bin(AluOp.BITWISE_NOT, d, d)
    y0 = nx * C0
    recip2 = Spec(body=y0 * (C1 - d * y0), reference=_ref_recip2)

    # RECIP1PT2: out = recip1(1 + Src0^2); accum = sum  (corr: sum 1/(1+t^2))
    d2 = Src0 * Src0 + One
    nx2 = Bin(AluOp.BITWISE_NOT, d2, d2)
    y02 = nx2 * C0
    recip1pt2 = Spec(body=y02 * (C1 - d2 * y02), accum=AluOp.ADD,
                     accum_init=Zero, reference=_ref_recip1pt2)

    # WT: out = Src0*C0 + Src1*C1  (C0/C1 = per-partition scalar APs Sq, Sp)
    wt = Spec(body=Src0 * C0 + Src1 * C1, reference=_ref_wt)

    specs = {"ANT_CSI_SQSUM": sqsum, "ANT_CSI_RECIP2": recip2,
             "ANT_CSI_RECIP1PT2": recip1pt2, "ANT_CSI_WT": wt}
    ops = {}
    for name, spec in specs.items():
        if name in _SUB_OPCODE_FOR_NAME:
            ops[name] = next(o for o in OPS if o.name == name)
            continue
        row = _CUSTOM_DVE_ROW_BASE + len(OPS)
        sha = {}
        for ver in ("v3", "v4"):
            try:
                s = DveOpSpec(name=name, opcode=row, uops=lower(spec, ver=ver))
                sha[ver] = s.sha(ver)
            except Exception:
                pass
        op = DveOp(name, spec, subdim=False, uops_sha=sha)
        OPS.append(op)
        CUSTOM_DVE_SPECS[name] = spec
        _SUB_OPCODE_FOR_NAME[name] = row
        ops[name] = op
    return ops


_OPS = _make_ops()
OP_SQSUM = _OPS["ANT_CSI_SQSUM"]
OP_RECIP2 = _OPS["ANT_CSI_RECIP2"]
OP_RECIP1PT2 = _OPS["ANT_CSI_RECIP1PT2"]
OP_WT = _OPS["ANT_CSI_WT"]


# ---------------------------------------------------------------------------
class Sched:
    """Dependency scheduler for raw Bass (per-engine streams + semaphores)."""

    def __init__(self, nc):
        self.nc = nc
        self.ops = []
        self.cum = {e: 0 for e in _ENGINES}
        self.writer = {}
        self.readers = {}

    def add(self, engine, fn, reads=(), writes=(), inc=1):
        idx = len(self.ops)
        deps = set()
        for s in reads:
            w = self.writer.get(s)
            if w is not None:
                deps.add(w)
        for s in writes:
            for rd in self.readers.get(s, ()):
                deps.add(rd)
            w = self.writer.get(s)
            if w is not None:
                deps.add(w)
        self.cum[engine] += inc
        self.ops.append(dict(engine=engine, fn=fn, deps=deps, inc=inc,
                             cum=self.cum[engine], idx=idx))
        for s in reads:
            self.readers.setdefault(s, []).append(idx)
        for s in writes:
            self.writer[s] = idx
            self.readers[s] = []
        return idx

    def emit(self):
        nc = self.nc
        sems = {e: nc.alloc_semaphore(name=f"sem_{e}") for e in _ENGINES}
        streams = {s: [op for op in self.ops if _STREAM_OF[op["engine"]] == s]
                   for s in ("sync", "vector", "scalar", "gpsimd")}
        waited = {s: {p: 0 for p in _ENGINES}
                  for s in ("sync", "vector", "scalar", "gpsimd")}

        def run_stream(eng_handle, stream):
            for op in streams[stream]:
                need = {}
                for d in op["deps"]:
                    dop = self.ops[d]
                    pe = dop["engine"]
                    if _STREAM_OF[pe] == stream and pe == op["engine"]:
                        continue
                    need[pe] = max(need.get(pe, 0), dop["cum"])
                for pe, val in need.items():
                    if val > waited[stream][pe]:
                        eng_handle.wait_ge(sems[pe], val)
                        waited[stream][pe] = val
                inst = op["fn"]()
                inst.then_inc(sems[op["engine"]], op["inc"])

        with nc.Block() as block:
            @block.sync
            def _(sync):
                run_stream(sync, "sync")

            @block.vector
            def _(vector):
                run_stream(vector, "vector")

            @block.scalar
            def _(scalar):
                run_stream(scalar, "scalar")

            @block.gpsimd
            def _(gpsimd):
                run_stream(gpsimd, "gpsimd")

            total_s = self.cum["sync"]

            @block.gpsimd
            def _(gpsimd):
                gpsimd.wait_ge(sems["sync"], total_s)


# ---------------------------------------------------------------------------
def build_kernel(nblk=NBLK, reps=1):
    """reps>1 repeats the whole program (same data, same acc cols) for
    slope-based HW timing; results are identical to reps=1."""
    nc = bass.Bass(trn_type="TRN2")
    rows = nblk * 128

    # const AP for activation bias 0.0
    c0 = nc.alloc_sbuf_tensor("const-zero", [128, 1], F32)
    nc.gpsimd.memset(c0.ap(), 0.0)
    nc.const_aps.aps[(F32, 0.0)] = c0.ap()
    nc.all_engine_barrier()

    ins = {nm: nc.dram_tensor(nm, [rows, N], F32, kind="ExternalInput")
           for nm in ("pred_re", "pred_im", "target_re", "target_im")}
    nchunks = nblk * NCH
    acc_cols = nchunks * NSTAT
    acc_out = nc.dram_tensor("acc_out", [128, acc_cols], F32,
                             kind="ExternalOutput")

    def btile(nm, nbuf, w, dt=BF16):
        return [nc.alloc_sbuf_tensor(f"{nm}{i}", [128, w], dt).ap()
                for i in range(nbuf)]

    # block input tiles (bf16, full 4096 wide), double buffered
    a1B = btile("a1B", 2, N); b1B = btile("b1B", 2, N)
    a2B = btile("a2B", 2, N); b2B = btile("b2B", 2, N)
    # chunk tiles
    p2T = btile("p2", 2, CHUNK); q2T = btile("q2", 2, CHUNK)
    lnpT = btile("lnp", 2, CHUNK); lnqT = btile("lnq", 2, CHUNK)
    uT = btile("u", 2, CHUNK); vT = btile("v", 2, CHUNK)
    m1T = btile("m1", 1, CHUNK); m2T = btile("m2", 1, CHUNK)
    y1T = btile("y1", 1, CHUNK); y2T = btile("y2", 1, CHUNK)
    crT = btile("cr", 1, CHUNK); snT = btile("sn", 1, CHUNK)
    uvT = btile("uv", 1, CHUNK); idenT = btile("iden", 1, CHUNK)
    ratT = btile("rat", 2, CHUNK); ph2T = btile("ph2", 2, CHUNK)
    wtT = btile("wt", 4, CHUNK); lwT = btile("lw", 4, CHUNK)
    junkT = btile("junk", 1, CHUNK); sinT = btile("sinp", 1, CHUNK)

    acc = nc.alloc_sbuf_tensor("acc", [128, acc_cols], F32).ap()
    lil = nc.alloc_sbuf_tensor("lil", [128, 2 * nblk], F32).ap()

    sch = Sched(nc)

    def A(g, s):
        i = g * NSTAT + s
        return acc[:, i:i + 1], f"acc{i}"

    in_names = ("pred_re", "pred_im", "target_re", "target_im")
    in_tiles = (a1B, b1B, a2B, b2B)

    # Deferred wt-tail ops (lw + Wacc) from the previous block, flushed
    # inside the next block's ln/exp Act section so the Act stream is
    # [lnexp x10][trig x4] per block: 2 table loads instead of 4.
    pending_tail = []

    for gblk in range(nblk * reps):
        bkl = gblk % nblk
        ib = gblk % 2
        bq = (gblk % 2) * 2  # wt/lw buffer base for this block
        r0 = bkl * 128
        # casting loads (gpsimd SWDGE), one per tensor per block
        ld_idxs = []
        for nm, tl in zip(in_names, in_tiles):
            src = ins[nm][r0:r0 + 128, :]
            ld_idxs.append(sch.add(
                f"gdma{ib}",
                lambda d=tl[ib], s=src: nc.gpsimd.dma_start(d[:], s),
                writes=(f"{nm}B{ib}",), inc=16))
        # consumers must wait for the whole 4-load set (completions are
        # unordered within the set)
        for i in ld_idxs:
            sch.ops[i]["cum"] = sch.cum[f"gdma{ib}"]

        rb = (f"pred_reB{ib}", f"pred_imB{ib}",
              f"target_reB{ib}", f"target_imB{ib}")

        def chunk_aps(c):
            cs = slice(c * CHUNK, (c + 1) * CHUNK)
            return (a1B[ib][:, cs], b1B[ib][:, cs],
                    a2B[ib][:, cs], b2B[ib][:, cs])

        # --- phase 1 (per chunk): squares + ln/exp (Act: natural_log_exp)
        for c in range(NCH):
            g = bkl * NCH + c
            p = g % 2
            a1, b1, a2, b2 = chunk_aps(c)
            aap, asl = A(g, S_UU)
            sch.add("vector", lambda o=p2T[p], i0=a1, i1=b1, aa=aap:
                    nc.vector._custom_dve(OP_SQSUM, out=o[:], in0=i0, in1=i1,
                                          s0=1e-30, s1=0.0, accum_out=aa),
                    reads=(rb[0], rb[1]), writes=(f"p2{p}", asl))
            aap, asl = A(g, S_VV)
            sch.add("vector", lambda o=q2T[p], i0=a2, i1=b2, aa=aap:
                    nc.vector._custom_dve(OP_SQSUM, out=o[:], in0=i0, in1=i1,
                                          s0=1e-30, s1=0.0, accum_out=aa),
                    reads=(rb[2], rb[3]), writes=(f"q2{p}", asl))
            sch.add("scalar", lambda o=lnpT[p], i=p2T[p]:
                    nc.scalar.activation(o[:], i[:], AF.Ln),
                    reads=(f"p2{p}",), writes=(f"lnp{p}",))
            sch.add("scalar", lambda o=lnqT[p], i=q2T[p]:
                    nc.scalar.activation(o[:], i[:], AF.Ln),
                    reads=(f"q2{p}",), writes=(f"lnq{p}",))
            aap, asl = A(g, S_U)
            sch.add("scalar", lambda o=uT[p], i=lnpT[p], aa=aap:
                    nc.scalar.activation(o[:], i[:], AF.Exp, scale=0.5,
                                         accum_out=aa),
                    reads=(f"lnp{p}",), writes=(f"u{p}", asl))
            aap, asl = A(g, S_V)
            sch.add("scalar", lambda o=vT[p], i=lnqT[p], aa=aap:
                    nc.scalar.activation(o[:], i[:], AF.Exp, scale=0.5,
                                         accum_out=aa),
                    reads=(f"lnq{p}",), writes=(f"v{p}", asl))

        # --- flush previous block's wt tail (lw is ln table: no switch)
        for fn in pending_tail:
            fn()
        pending_tail = []

        # --- phase 2 (per chunk): cross products, phase ratio, arctan
        for c in range(NCH):
            g = bkl * NCH + c
            p = g % 2
            a1, b1, a2, b2 = chunk_aps(c)
            sch.add("vector", lambda o=m1T[0], i0=a1, i1=a2:
                    nc.vector.tensor_tensor(out=o[:], in0=i0, in1=i1,
                                            op=ALU.mult),
                    reads=(rb[0], rb[2]), writes=("m1",))
            sch.add("gpsimd", lambda o=m2T[0], i0=b1, i1=b2:
                    nc.gpsimd.tensor_tensor(out=o[:], in0=i0, in1=i1,
                                            op=ALU.mult),
                    reads=(rb[1], rb[3]), writes=("m2",))
            sch.add("gpsimd", lambda o=crT[0], i0=m1T[0], i1=m2T[0]:
                    nc.gpsimd.tensor_tensor(out=o[:], in0=i0[:], in1=i1[:],
                                            op=ALU.add),
                    reads=("m1", "m2"), writes=("cr",))
            sch.add("vector", lambda o=y1T[0], i0=b1, i1=a2:
                    nc.vector.tensor_tensor(out=o[:], in0=i0, in1=i1,
                                            op=ALU.mult),
                    reads=(rb[1], rb[2]), writes=("y1",))
            sch.add("gpsimd", lambda o=y2T[0], i0=a1, i1=b2:
                    nc.gpsimd.tensor_tensor(out=o[:], in0=i0, in1=i1,
                                            op=ALU.mult),
                    reads=(rb[0], rb[3]), writes=("y2",))
            sch.add("gpsimd", lambda o=snT[0], i0=y1T[0], i1=y2T[0]:
                    nc.gpsimd.tensor_tensor(out=o[:], in0=i0[:], in1=i1[:],
                                            op=ALU.subtract),
                    reads=("y1", "y2"), writes=("sn",))

            aap, asl = A(g, S_UV)
            sch.add("vector", lambda o=uvT[0], i0=uT[p], i1=vT[p], aa=aap:
                    nc.vector._custom_dve(OP_TTR, out=o[:], in0=i0[:],
                                          in1=i1[:], s0=0.0, s1=1.0,
                                          accum_out=aa),
                    reads=(f"u{p}", f"v{p}"), writes=("uv", asl))
            sch.add("vector", lambda o=idenT[0], i0=uvT[0], i1=crT[0]:
                    nc.vector._custom_dve(OP_RECIP2, out=o[:], in0=i0[:],
                                          in1=i1[:], s0=RCP_C0, s1=RCP_C1,
                                          imm2=1e-9),
                    reads=("uv", "cr"), writes=("iden",))
            sch.add("vector", lambda o=ratT[p], i0=snT[0], i1=idenT[0]:
                    nc.vector.tensor_tensor(out=o[:], in0=i0[:], in1=i1[:],
                                            op=ALU.mult),
                    reads=("sn", "iden"), writes=(f"rat{p}",))

            # Act trig section: ph2 = Arctan(rat); S_PH += (2*ph2)^2;
            # corr via sn2 = Sin(ph2), S_IC += sin^2 (cos dth = 1 - 2 sin^2)
            sch.add("scalar", lambda o=ph2T[p], i=ratT[p]:
                    nc.scalar.activation(o[:], i[:], AF.Arctan),
                    reads=(f"rat{p}",), writes=(f"ph2{p}",))
            aap, asl = A(g, S_PH)
            sch.add("scalar", lambda o=junkT[0], i=ph2T[p], aa=aap:
                    nc.scalar.activation(o[:], i[:], AF.Square, scale=2.0,
                                         accum_out=aa),
                    reads=(f"ph2{p}",), writes=("junks", asl))
            sch.add("scalar", lambda o=sinT[0], i=ph2T[p]:
                    nc.scalar.activation(o[:], i[:], AF.Sin),
                    reads=(f"ph2{p}",), writes=("sinp",))
            aap, asl = A(g, S_IC)
            sch.add("scalar", lambda o=junkT[0], i=sinT[0], aa=aap:
                    nc.scalar.activation(o[:], i[:], AF.Square,
                                         accum_out=aa),
                    reads=("sinp",), writes=("junks", asl))

            aap, asl = A(g, S_R8)
            sch.add("vector", lambda o=junkT[0], i0=uT[p], i1=lnpT[p], aa=aap:
                    nc.vector._custom_dve(OP_TTR, out=o[:], in0=i0[:],
                                          in1=i1[:], s0=0.0, s1=0.5,
                                          accum_out=aa),
                    reads=(f"u{p}", f"lnp{p}"), writes=("junk", asl))
            aap, asl = A(g, S_R9)
            sch.add("vector", lambda o=junkT[0], i0=vT[p], i1=lnqT[p], aa=aap:
                    nc.vector._custom_dve(OP_TTR, out=o[:], in0=i0[:],
                                          in1=i1[:], s0=0.0, s1=0.5,
                                          accum_out=aa),
                    reads=(f"v{p}", f"lnq{p}"), writes=("junk", asl))

        # --- per-block: Sp = S_U(c0)+S_U(c1), Sq likewise. On Pool: the WT
        # custom op's scalar port reads lil at instruction ISSUE, so the
        # producer must be on another engine (cross-engine sem guarantees
        # the write is visible); a same-engine back-to-back write races.
        g0, g1 = bkl * NCH, bkl * NCH + 1
        lu = lil[:, 2 * bkl:2 * bkl + 1]
        lv = lil[:, 2 * bkl + 1:2 * bkl + 2]
        sch.add("gpsimd", lambda o=lu, i0=A(g0, S_U)[0], i1=A(g1, S_U)[0]:
                nc.gpsimd.tensor_tensor(out=o, in0=i0, in1=i1, op=ALU.add),
                reads=(A(g0, S_U)[1], A(g1, S_U)[1]), writes=(f"lu{bkl}",))
        sch.add("gpsimd", lambda o=lv, i0=A(g0, S_V)[0], i1=A(g1, S_V)[0]:
                nc.gpsimd.tensor_tensor(out=o, in0=i0, in1=i1, op=ALU.add),
                reads=(A(g0, S_V)[1], A(g1, S_V)[1]), writes=(f"lv{bkl}",))

        # --- wt = Sq*u + Sp*v now; Ln(wt) + W accum deferred to next block
        for c in range(NCH):
            g = bkl * NCH + c
            p = g % 2
            q = bq + c
            sch.add("vector", lambda o=wtT[q], i0=uT[p], i1=vT[p],
                    s0=lv, s1=lu:
                    nc.vector._custom_dve(OP_WT, out=o[:], in0=i0[:],
                                          in1=i1[:], s0=s0, s1=s1),
                    reads=(f"u{p}", f"v{p}", f"lu{bkl}", f"lv{bkl}"),
                    writes=(f"wt{q}",))

            def tail(g=g, p=p, q=q):
                sch.add("scalar", lambda o=lwT[q], i=wtT[q]:
                        nc.scalar.activation(o[:], i[:], AF.Ln),
                        reads=(f"wt{q}",), writes=(f"lw{q}",))
                aap, asl = A(g, S_W)
                sch.add("vector", lambda o=junkT[0], i0=wtT[q], i1=lwT[q],
                        aa=aap:
                        nc.vector._custom_dve(OP_TTR, out=o[:], in0=i0[:],
                                              in1=i1[:], s0=0.0, s1=1.0,
                                              accum_out=aa),
                        reads=(f"wt{q}", f"lw{q}"), writes=("junk", asl))

            pending_tail.append(tail)

    for fn in pending_tail:
        fn()
    pending_tail = []

    all_acc = tuple(f"acc{i}" for i in range(acc_cols))
    sch.add("sync", lambda: nc.sync.dma_start(acc_out[:, :], acc[:, :]),
            reads=all_acc, writes=(), inc=16)

    sch.emit()
    # Bacc.compile()-equivalent finalization for the raw-Bass path:
    # gpsimd library loads (Pool SW ops execute garbage without them) and
    # typed-ISA instruction encoding (walrus rejects ISA len 0 otherwise).
    import bass_rust as _bass_rust
    from concourse.library_config import all_libraries, standard
    inst_type_to_lib_mask = {}
    for lib_i, lib in enumerate(all_libraries):
        for inst_type in lib.instructions:
            inst_type_to_lib_mask[inst_type] = (
                inst_type_to_lib_mask.get(inst_type, 0) | (1 << lib_i))
    _bass_rust.insert_library_loads(
        nc, inst_type_to_lib_mask, len(all_libraries), standard.index)
    mybir.codegen_inst_isa_subclasses(nc)
    return nc


# ---------------------------------------------------------------------------
def _host_reduce(accs, nblk=NBLK):
    """accs: list of per-core [128, nchunks*NSTAT] f32 -> final loss."""
    nchunks = nblk * NCH
    rows_per_core = nblk * 128
    stats = np.zeros((len(accs) * rows_per_core, NSTAT), np.float64)
    for k, a in enumerate(accs):
        a = a.astype(np.float64)
        for bkl in range(nblk):
            rows = slice(k * rows_per_core + bkl * 128,
                         k * rows_per_core + (bkl + 1) * 128)
            tot = np.zeros((128, NSTAT))
            for c in range(NCH):
                col0 = (bkl * NCH + c) * NSTAT
                tot += a[:, col0:col0 + NSTAT]
            stats[rows] = tot
    nrows = stats.shape[0]
    s_uu, s_vv = stats[:, S_UU], stats[:, S_VV]
    s_u, s_v, s_uv = stats[:, S_U], stats[:, S_V], stats[:, S_UV]
    s_ph, s_ic = stats[:, S_PH], stats[:, S_IC]
    r8, r9, W = stats[:, S_R8], stats[:, S_R9], stats[:, S_W]

    n = float(N)
    total = float(nrows) * n
    mag_loss = (s_uu - 2 * s_uv + s_vv).sum() / total
    p_mean, t_mean = s_u / n, s_v / n
    mean_loss = ((p_mean - t_mean) ** 2).mean()
    p_var = np.clip(s_uu / n - p_mean ** 2, 1e-12, None)
    t_var = np.clip(s_vv / n - t_mean ** 2, 1e-12, None)
    std_loss = ((np.sqrt(p_var) - np.sqrt(t_var)) ** 2).mean()
    # S_PH holds sum (2*ph2)^2 = sum dth^2 (Act Square with scale=2)
    phase_loss = s_ph.sum() / total
    # S_IC holds sum sin^2(ph2); cos(dth) = 1 - 2 sin^2(dth/2)
    cos_total = total - 2.0 * s_ic.sum()
    corr_loss = 2.0 - 2.0 * cos_total / total
    # r8/r9 are 0.5 * sum u*ln(p2) = sum u ln u (TTR scale=0.5)
    r8h, r9h = r8, r9
    js = 0.5 * (r8h / s_u + r9h / s_v - W / (s_u * s_v)
                + np.log(s_u) + np.log(s_v) + 2 * np.log(2.0))
    js_loss = js.mean()
    loss = (0.5 * mag_loss + 0.25 * mean_loss + 0.15 * std_loss
            + 0.5 * phase_loss + 0.2 * corr_loss + 0.1 * js_loss)
    return loss


_NC_CACHE = None


def _get_nc():
    global _NC_CACHE
    if _NC_CACHE is None:
        _NC_CACHE = build_kernel()
    return _NC_CACHE


def kernel(pred_re, pred_im, target_re, target_im, _trace=False):
    nc = _get_nc()
    arrs = {"pred_re": pred_re, "pred_im": pred_im,
            "target_re": target_re, "target_im": target_im}
    in_maps = []
    for k in range(NCORES):
        rows = slice(k * ROWS_PER_CORE, (k + 1) * ROWS_PER_CORE)
        in_maps.append({nm: np.ascontiguousarray(
            np.asarray(a)[rows], dtype=np.float32) for nm, a in arrs.items()})
    res = run_bass_kernel_spmd(nc, in_maps, core_ids=list(range(NCORES)),
                               trace=_trace)
    accs = [res.results[k]["acc_out"] for k in range(NCORES)]
    loss = _host_reduce(accs)
    out = np.float32(loss)
    if _trace:
        return out, res
    return out
